# revision 6
# baseline (speedup 1.0000x reference)
"""Multi-head attention (B=4, T=2048, C=1024, H=16, D=64) on 8 TRN2 cores.

Sharding: core i handles batch b=i//2 and heads of half hh=i%2 (8 heads =
4 pairs). Row-sharded output projection -> partial y [T, C]; host sums the
two partials per batch.

Dataflow (per core):
  q/k projections: fp8e4 DoubleRow matmuls (weights prescaled x32 on host;
    DR contracts 2x128 c-chunks per pass -> half the PE rows), psum f32
    cast to persistent fp8 qk8[pair][q/k] [(u d), t] tiles by DVE.
  scores: one fp8 matmul per (s-block, head): lhsT=kT [64,128],
    rhs=qT [64, N], N trimmed to the causal block range.
  exp on ScalarE psum->bf16 with scale 1/32768 (1/sqrt(C)/32/32); diagonal
    blocks zeroed post-exp on GpSimd (affine_select, t<s -> 0).
  PV: bf16, lhsT=[v|ones] [128,65], transposed accumulate; row 64 = Z.
  normalize: Z rows copied to bf16, Z broadcast via ones-matmul into psum,
    reciprocal_approx_fast on the broadcast (DVE), multiply on GpSimd.
  y: bf16 matmuls over 4 pair-chunks + DVE bias add, DMA out f32.
  Weave: projection/v/y pieces are emitted as fillers inside the ACT-bound
  attention stream to keep the PE busy; y(j) unlocks after pair 3's
  per-j normalize and weaves into the following attention tile.
"""

import os
import sys
from collections import deque

import numpy as np
import ml_dtypes

for _p in ("/opt/trn_rl_repo", "/root/.axon_site/_ro/trn_rl_repo"):
    if os.path.isdir(_p) and _p not in sys.path:
        sys.path.append(_p)

import concourse.bass as bass
import concourse.bacc as bacc
import concourse.mybir as mybir
import concourse.tile as tile
from concourse.bass_utils import run_bass_kernel_spmd

B, T, C, H, D = 4, 2048, 1024, 16, 64
HL = H // 2          # heads per core
P = 128
NTT = T // 512       # 4 t-tiles of 512
NSB = T // P         # 16 s-blocks of 128
SCALE_DR = 1.0 / 32768.0   # 1/sqrt(C) / (32*32)

F32 = mybir.dt.float32
F32R = mybir.dt.float32r
BF16 = mybir.dt.bfloat16
F8 = mybir.dt.float8e4
DR = mybir.MatmulPerfMode.DoubleRow
EXP = mybir.ActivationFunctionType.Exp

E4 = ml_dtypes.float8_e4m3
BF = ml_dtypes.bfloat16


def _build(causal: bool) -> bass.Bass:
    nc = bacc.Bacc("TRN2", target_bir_lowering=False, debug=False, num_devices=8)

    x8_d = nc.dram_tensor("x8", [C, T], F8, kind="ExternalInput").ap()
    x16_d = nc.dram_tensor("x16", [C, T], BF16, kind="ExternalInput").ap()
    wq8_d = nc.dram_tensor("wq8", [C, HL * D], F8, kind="ExternalInput").ap()
    wk8_d = nc.dram_tensor("wk8", [C, HL * D], F8, kind="ExternalInput").ap()
    wv16_d = nc.dram_tensor("wv16", [C, HL * D], BF16, kind="ExternalInput").ap()
    wpt16_d = nc.dram_tensor("wpt16", [HL * D, C], BF16, kind="ExternalInput").ap()
    bp_d = nc.dram_tensor("bp", [C], F32, kind="ExternalInput").ap()
    y_d = nc.dram_tensor("y", [T, C], F32, kind="ExternalOutput").ap()

    with tile.TileContext(nc) as tc:
        _emit(nc, tc, causal, x8_d, x16_d, wq8_d, wk8_d, wv16_d, wpt16_d,
              bp_d, y_d)
    nc.compile()
    return nc


def _emit(nc, tc, causal, x8_d, x16_d, wq8_d, wk8_d, wv16_d, wpt16_d,
          bp_d, y_d):
    from contextlib import ExitStack

    ctx = ExitStack()
    with ctx:
        x8_pool = ctx.enter_context(tc.tile_pool(name="x8", bufs=1))
        x16_pool = ctx.enter_context(tc.tile_pool(name="x16", bufs=1))
        w_pool = ctx.enter_context(tc.tile_pool(name="w", bufs=1))
        qkdr_pool = ctx.enter_context(tc.tile_pool(name="qkdr", bufs=8))
        stage_pool = ctx.enter_context(tc.tile_pool(name="stage", bufs=2))
        v_pool = ctx.enter_context(tc.tile_pool(name="v16", bufs=4))
        oc_pool = ctx.enter_context(tc.tile_pool(name="outcat", bufs=4))
        p_pool = ctx.enter_context(tc.tile_pool(name="pts", bufs=4))
        z_pool = ctx.enter_context(tc.tile_pool(name="zb", bufs=2))
        rz_pool = ctx.enter_context(tc.tile_pool(name="rz", bufs=2))
        bps_pool = ctx.enter_context(tc.tile_pool(name="bps", bufs=2))
        yst_pool = ctx.enter_context(tc.tile_pool(name="yst", bufs=3))
        psS = ctx.enter_context(tc.tile_pool(name="psS", bufs=2, space="PSUM"))
        psO = ctx.enter_context(tc.tile_pool(name="psO", bufs=2, space="PSUM"))
        psA = ctx.enter_context(tc.tile_pool(name="psA", bufs=2, space="PSUM"))

        # ---- weight / bias / x DMAs (proj-p0 critical path first) ----
        wq8_t = w_pool.tile([P, 8, HL * D], F8, tag="wq8")
        wk8_t = w_pool.tile([P, 8, HL * D], F8, tag="wk8")
        wv16_t = w_pool.tile([P, 8, HL * D], BF16, tag="wv16")
        wpt16_t = w_pool.tile([P, 4, C], BF16, tag="wpt16")
        x8_t = x8_pool.tile([P, 8, T], F8, tag="x8")
        x16_t = x16_pool.tile([P, 8, T], BF16, tag="x16")

        for w_t, w_d in ((wq8_t, wq8_d), (wk8_t, wk8_d)):
            nc.sync.dma_start(
                out=w_t, in_=w_d.rearrange("(n p) d -> p n d", p=P))
        for ch in range(8):
            for th in range(2):
                tg = slice(th * 1024, (th + 1) * 1024)
                nc.sync.dma_start(
                    out=x8_t[:, ch, tg],
                    in_=x8_d[ch * P:(ch + 1) * P, tg])
        nc.sync.dma_start(
            out=wv16_t, in_=wv16_d.rearrange("(n p) d -> p n d", p=P))
        for ch in range(8):
            for th in range(2):
                tg = slice(th * 1024, (th + 1) * 1024)
                nc.sync.dma_start(
                    out=x16_t[:, ch, tg],
                    in_=x16_d[ch * P:(ch + 1) * P, tg])
        nc.sync.dma_start(
            out=wpt16_t, in_=wpt16_d.rearrange("(n p) d -> p n d", p=P))
        bpb = w_pool.tile([P, C], F32, tag="bpb")
        nc.sync.dma_start(
            out=bpb,
            in_=bass.AP(tensor=bp_d.tensor, offset=0, ap=[[0, P], [1, C]]))

        # persistent fp8 q/k per pair: [(u d), t]
        qk8 = [[qkdr_pool.tile([P, T], F8, tag="qk8", name=f"qk8_{pr}_{qk}")
                for qk in range(2)] for pr in range(4)]

        # v: 4 groups of 4 s-blocks: [s-part, sb%4, head, d | ones]
        v16_g = [v_pool.tile([P, 4, HL, D + 1], BF16, tag="v16",
                             name=f"v16_{g}") for g in range(4)]
        for g in range(4):
            nc.vector.memset(v16_g[g][:, :, :, D:], 1.0)

        outcat = [oc_pool.tile([P, T], BF16, tag="outcat", name=f"oc{q}")
                  for q in range(4)]

        ones_bc16 = w_pool.tile([1, P], BF16, tag="ones")
        nc.vector.memset(ones_bc16, 1.0)

        # ---------- emit helpers ----------
        def emit_proj_piece(pr, th, qk, tt):
            w_t = wq8_t if qk == 0 else wk8_t
            stage = qk8[pr][qk]
            ps = psA.tile([P, 512], F32, tag="psA", name="qkps")
            for sub in range(2):
                n0 = th * 1024 + tt * 512 + sub * 256
                for cp in range(4):
                    nc.tensor.matmul(
                        ps[:, sub * 256:(sub + 1) * 256],
                        w_t[:, 2 * cp:2 * cp + 2, pr * P:(pr + 1) * P],
                        x8_t[:, 2 * cp:2 * cp + 2, n0:n0 + 256],
                        start=(cp == 0), stop=(cp == 3), perf_mode=DR)
            nc.vector.tensor_copy(
                out=stage[:, th * 1024 + tt * 512:
                          th * 1024 + (tt + 1) * 512], in_=ps)

        def emit_v_piece(sb):
            ps = psA.tile([P, 512], F32, tag="psA", name="vps")
            for ch in range(8):
                nc.tensor.matmul(
                    ps, x16_t[:, ch, sb * P:(sb + 1) * P], wv16_t[:, ch, :],
                    start=(ch == 0), stop=(ch == 7))
            nc.vector.tensor_copy(
                out=v16_g[sb // 4][:, sb % 4, :, 0:D],
                in_=ps.rearrange("p (h d) -> p h d", h=HL))

        def emit_y_piece(m, n):
            yps = psA.tile([P, 512], F32, tag="psA", name="yps")
            for q in range(4):
                nc.tensor.matmul(
                    yps, outcat[q][:, m * P:(m + 1) * P],
                    wpt16_t[:, q, n * 512:(n + 1) * 512],
                    start=(q == 0), stop=(q == 3))
            yt = yst_pool.tile([P, 512], F32, tag="yst")
            nc.vector.tensor_add(yt, yps, bpb[:, n * 512:(n + 1) * 512])
            nc.sync.dma_start(
                out=y_d[m * P:(m + 1) * P, n * 512:(n + 1) * 512], in_=yt)

        fillers = deque()

        def pump(k=1):
            for _ in range(k):
                if fillers:
                    fillers.popleft()()

        def drain():
            while fillers:
                fillers.popleft()()

        def emit_norm(pr, j, outp):
            # Z rows (bf16) -> broadcast Z via bf16 ones-matmul -> reciprocal
            # of the broadcast on DVE -> normalize multiply on GpSimd.
            zbb = z_pool.tile([1, 2, 512], BF16, tag="zb")
            for u in range(2):
                nc.vector.tensor_copy(out=zbb[:, u, :], in_=outp[u][D:D + 1, :])
            bzp = psA.tile([P, 512], F32, tag="psA", name="bzp")
            for u in range(2):
                nc.tensor.matmul(
                    bzp[u * D:(u + 1) * D, :], ones_bc16[:, 0:D],
                    zbb[:, u, :], start=True, stop=True,
                    tile_position=(0, u * D))
            bz = bps_pool.tile([P, 512], F32, tag="bps")
            nc.vector.reciprocal_approx_fast(out=bz, in_=bzp)
            for u in range(2):
                osl = outcat[pr][u * D:(u + 1) * D, j * 512:(j + 1) * 512]
                nc.vector.tensor_copy(out=osl, in_=outp[u][0:D, :])
                nc.gpsimd.tensor_mul(osl, osl, bz[u * D:(u + 1) * D, :])

        def emit_attention_pair(pr):
            for j in range(NTT):
                nsb_j = 4 * (j + 1) if causal else NSB
                outp = [psO.tile([D + 1, 512], F32, tag="psO",
                                 name=f"outp{pr}_{j}_{u}") for u in range(2)]

                def emit_pv(i, lo, last):
                    pts = pend.pop(i)
                    for u in range(2):
                        nc.tensor.matmul(
                            outp[u][:, lo:512],
                            v16_g[i // 4][:, i % 4, pr * 2 + u, :],
                            pts[:, u, lo:512],
                            start=(i == 0), stop=last,
                            skip_group_check=True)

                pend = {}
                prev = None
                for i in range(nsb_j):
                    r = i - 4 * j if causal else -1
                    lo = max(r, 0) * P
                    scs = psS.tile([P, 2, 512], F32, tag="scs")
                    pts = p_pool.tile([P, 2, 512], BF16, tag="pts")
                    pend[i] = pts
                    for u in range(2):
                        dsl = slice(u * D, (u + 1) * D)
                        nc.tensor.matmul(
                            scs[:, u, lo:512],
                            qk8[pr][1][dsl, i * P:(i + 1) * P],
                            qk8[pr][0][dsl, j * 512 + lo:(j + 1) * 512],
                            start=True, stop=True)
                    nc.scalar.activation(
                        out=pts[:, :, lo:512], in_=scs[:, :, lo:512],
                        func=EXP, scale=SCALE_DR)
                    if causal and r >= 0:
                        nc.gpsimd.affine_select(
                            out=pts[:, :, lo:lo + P],
                            in_=pts[:, :, lo:lo + P],
                            compare_op=mybir.AluOpType.is_ge,
                            fill=0.0, base=0,
                            pattern=[[0, 2], [1, P]], channel_multiplier=-1)
                    if prev is not None:
                        emit_pv(*prev)
                        pump(1)
                    prev = (i, lo, i == nsb_j - 1)
                emit_pv(*prev)
                pump(1)
                emit_norm(pr, j, outp)
                if pr == 3:
                    jj = j
                    for m in range(4 * jj, 4 * jj + 4):
                        for n in range(2):
                            fillers.append(
                                lambda m=m, n=n: emit_y_piece(m, n))

        # ---------- schedule ----------
        # proj pair 0 + first v group up front
        for th in range(2):
            for qk in range(2):
                for tt in range(2):
                    emit_proj_piece(0, th, qk, tt)
        for sb in range(4):
            emit_v_piece(sb)

        for pr in range(4):
            # queue fillers: remaining v groups (pair 0 only) then next proj
            if pr == 0:
                for sb in range(4, NSB):
                    fillers.append(lambda sb=sb: emit_v_piece(sb))
            if pr < 3:
                for th in range(2):
                    for qk in range(2):
                        for tt in range(2):
                            fillers.append(
                                lambda pr2=pr + 1, th=th, qk=qk, tt=tt:
                                emit_proj_piece(pr2, th, qk, tt))
            if pr > 0:
                # everything pair pr reads must be emitted before its reads
                drain()
            emit_attention_pair(pr)
        drain()


_NC_CACHE = {}
LAST_RESULTS = None


def kernel(x, Wq, Wk, Wv, Wp, bp, is_masked, **_unused):
    global LAST_RESULTS
    x = np.asarray(x, np.float32)
    Wq = np.asarray(Wq, np.float32)
    Wk = np.asarray(Wk, np.float32)
    Wv = np.asarray(Wv, np.float32)
    Wp = np.asarray(Wp, np.float32)
    bp = np.asarray(bp, np.float32)
    causal = bool(np.asarray(is_masked).item())

    if causal not in _NC_CACHE:
        _NC_CACHE[causal] = _build(causal)
    nc = _NC_CACHE[causal]

    wq_r = Wq.transpose(1, 0, 2).reshape(C, H * D)
    wk_r = Wk.transpose(1, 0, 2).reshape(C, H * D)
    wv_r = Wv.transpose(1, 0, 2).reshape(C, H * D)
    wpt = np.ascontiguousarray(Wp.T)
    zeros = np.zeros_like(bp)

    in_maps = []
    for core in range(8):
        b, hh = core // 2, core % 2
        csl = slice(hh * HL * D, (hh + 1) * HL * D)
        xT = np.ascontiguousarray(x[b].T)
        in_maps.append({
            "x8": xT.astype(E4),
            "x16": xT.astype(BF),
            "wq8": np.ascontiguousarray(32.0 * wq_r[:, csl]).astype(E4),
            "wk8": np.ascontiguousarray(32.0 * wk_r[:, csl]).astype(E4),
            "wv16": np.ascontiguousarray(wv_r[:, csl]).astype(BF),
            "wpt16": np.ascontiguousarray(wpt[csl, :]).astype(BF),
            "bp": bp if hh == 0 else zeros,
        })

    trace = bool(int(os.environ.get("KERNEL_TRACE", "0")))
    res = run_bass_kernel_spmd(
        nc, in_maps, core_ids=list(range(8)), trace=trace)
    LAST_RESULTS = res

    y = np.empty((B, T, C), np.float32)
    for b in range(B):
        y[b] = res.results[2 * b]["y"] + res.results[2 * b + 1]["y"]
    return y
